# revision 10
# baseline (speedup 1.0000x reference)
"""Trainium2 Bass kernel for nn_BATPoseDecoder.

Strategy: flatten (B,N) -> 1024 rows, 128 rows per core (8 cores, data
parallel; each core's rows live in a single batch b = core//2).

Per row r the device computes, reading L_row [256, 524] from DRAM once:
  scores = MLP(L_row)           (3-layer, fp32r matmuls, transposed layout)
  w      = softmax(scores)      (batched per 16 rows)
  matched = w^T @ L_row, tgt_matched = w^T @ tgt_b
  overlap = sigmoid(relu(matched@ow1+ob1)@ow2+ob2)  (batched per 16 rows)

Device outputs per core: corr_weights rows [128,256], overlap [1,128],
tgt_matched [128,3].  Host does the tiny O(B*N) weighted reductions +
3x3 SVD + quaternion tail.
"""
import numpy as np

import concourse.bass as bass
import concourse.bacc as bacc
import concourse.tile as tile
from concourse import mybir
from concourse.bass_utils import run_bass_kernel_spmd

F32 = mybir.dt.float32
F32R = mybir.dt.float32r
AF = mybir.ActivationFunctionType
AX = mybir.AxisListType
OP = mybir.AluOpType

B, N, M, D = 4, 256, 256, 524
ROWS = 128          # rows per core
NCORES = 8
GRP = 16            # rows per softmax/overlap group
DOF = [0, 128, 256, 384, 512]
DSZ = [128, 128, 128, 128, 12]
EPS = 1e-8

_CACHED_NC = None
DEBUG_DUMPS = False


def build_nc():
    nc = bacc.Bacc(None, target_bir_lowering=False, debug=False)

    lrows = nc.dram_tensor("lrows", [ROWS, M, D], F32, kind="ExternalInput")
    tgtb = nc.dram_tensor("tgtb", [128, 2, 4], F32, kind="ExternalInput")
    cw1p = nc.dram_tensor("cw1p", [5, 128, 256], F32, kind="ExternalInput")
    cb1 = nc.dram_tensor("cb1", [128, 2], F32, kind="ExternalInput")
    cw2 = nc.dram_tensor("cw2", [128, 2, 128], F32, kind="ExternalInput")
    cb2 = nc.dram_tensor("cb2", [128, 1], F32, kind="ExternalInput")
    cw3 = nc.dram_tensor("cw3", [128, 1], F32, kind="ExternalInput")
    cb3 = nc.dram_tensor("cb3", [1, 1], F32, kind="ExternalInput")
    ow1p = nc.dram_tensor("ow1p", [5, 128, 128], F32, kind="ExternalInput")
    ob1 = nc.dram_tensor("ob1", [128, 1], F32, kind="ExternalInput")
    ow2 = nc.dram_tensor("ow2", [128, 1], F32, kind="ExternalInput")
    ob2 = nc.dram_tensor("ob2", [1, 1], F32, kind="ExternalInput")
    ident = nc.dram_tensor("ident", [128, 128], F32, kind="ExternalInput")

    w_out = nc.dram_tensor("w_out", [ROWS, M], F32, kind="ExternalOutput")
    if DEBUG_DUMPS:
        dbg_xt = nc.dram_tensor("dbg_xt", [128, 5, 512], F32, kind="ExternalOutput")
        dbg_h1 = nc.dram_tensor("dbg_h1", [128, 2, 512], F32, kind="ExternalOutput")
        dbg_h2 = nc.dram_tensor("dbg_h2", [128, 512], F32, kind="ExternalOutput")
        dbg_s16 = nc.dram_tensor("dbg_s16", [GRP, 256], F32, kind="ExternalOutput")
    ov_out = nc.dram_tensor("ov_out", [1, ROWS], F32, kind="ExternalOutput")
    tm_out = nc.dram_tensor("tm_out", [ROWS, 3], F32, kind="ExternalOutput")

    with tile.TileContext(nc) as tc:
        from contextlib import ExitStack
        with ExitStack() as ctx:
            consts = ctx.enter_context(tc.tile_pool(name="consts", bufs=1))
            xpool = ctx.enter_context(tc.tile_pool(name="x", bufs=4))
            xrpool = ctx.enter_context(tc.tile_pool(name="xr", bufs=20))
            xtpool = ctx.enter_context(tc.tile_pool(name="xt", bufs=2))
            h1pool = ctx.enter_context(tc.tile_pool(name="h1", bufs=2))
            h2pool = ctx.enter_context(tc.tile_pool(name="h2", bufs=2))
            s16pool = ctx.enter_context(tc.tile_pool(name="s16", bufs=2))
            m16pool = ctx.enter_context(tc.tile_pool(name="m16", bufs=2))
            smpool = ctx.enter_context(tc.tile_pool(name="sm", bufs=4))
            outpool = ctx.enter_context(tc.tile_pool(name="outp", bufs=1))

            ps_xt = ctx.enter_context(
                tc.tile_pool(name="ps_xt", bufs=1, space="PSUM"))
            ps_h1 = ctx.enter_context(
                tc.tile_pool(name="ps_h1", bufs=1, space="PSUM"))
            ps_h2 = ctx.enter_context(
                tc.tile_pool(name="ps_h2", bufs=1, space="PSUM"))
            ps_rv = ctx.enter_context(
                tc.tile_pool(name="ps_rv", bufs=2, space="PSUM"))
            ps_gb = ctx.enter_context(
                tc.tile_pool(name="ps_gb", bufs=1, space="PSUM"))

            # ---------------- constants / weights preload ----------------
            id_f = consts.tile([128, 128], F32)
            nc.sync.dma_start(id_f[:], ident.ap())
            idr = consts.tile([128, 128], F32R)
            nc.vector.tensor_copy(idr[:], id_f[:])

            cw1_f = consts.tile([128, 5, 256], F32)
            nc.sync.dma_start(cw1_f[:], cw1p.ap().rearrange("t p f -> p t f"))
            cw1r = consts.tile([128, 5, 256], F32R)
            nc.vector.tensor_copy(cw1r[:], cw1_f[:])

            cw2_f = consts.tile([128, 2, 128], F32)
            nc.sync.dma_start(cw2_f[:], cw2.ap())
            cw2r = consts.tile([128, 2, 128], F32R)
            nc.vector.tensor_copy(cw2r[:], cw2_f[:])

            cw3_f = consts.tile([128, 1], F32)
            nc.sync.dma_start(cw3_f[:], cw3.ap())
            cw3r = consts.tile([128, 1], F32R)
            nc.vector.tensor_copy(cw3r[:], cw3_f[:])

            ow1_f = consts.tile([128, 5, 128], F32)
            nc.sync.dma_start(ow1_f[:], ow1p.ap().rearrange("t p f -> p t f"))
            ow1r = consts.tile([128, 5, 128], F32R)
            nc.vector.tensor_copy(ow1r[:], ow1_f[:])

            ow2_f = consts.tile([128, 1], F32)
            nc.sync.dma_start(ow2_f[:], ow2.ap())
            ow2r = consts.tile([128, 1], F32R)
            nc.vector.tensor_copy(ow2r[:], ow2_f[:])

            tgt_f = consts.tile([128, 2, 4], F32)
            nc.sync.dma_start(tgt_f[:], tgtb.ap())
            tgt_r = consts.tile([128, 2, 4], F32R)
            nc.vector.tensor_copy(tgt_r[:], tgt_f[:])

            cb1_sb = consts.tile([128, 2], F32)
            nc.sync.dma_start(cb1_sb[:], cb1.ap())
            cb2_sb = consts.tile([128, 1], F32)
            nc.sync.dma_start(cb2_sb[:], cb2.ap())
            cb3_sb = consts.tile([1, 1], F32)
            nc.sync.dma_start(cb3_sb[:], cb3.ap())
            ob1_sb = consts.tile([128, 1], F32)
            nc.sync.dma_start(ob1_sb[:], ob1.ap())
            ob2_sb = consts.tile([1, 1], F32)
            nc.sync.dma_start(ob2_sb[:], ob2.ap())

            ov_sb = outpool.tile([1, ROWS], F32)

            # gb: shared per-group psum bank:
            #   [:,0:32]    wT (softmax weights transposed, 2 m-chunks)
            #   [:,32:112]  matchedT (5 d-chunks x 16 rows)
            #   [:,112:128] h1o (overlap hidden)
            #   [0,128:144] h2o
            #   [0,144:156] matched d-tail chunk (per row)
            #   [0,156:159] tgt_matched (per row)
            gb = ps_gb.tile([128, 512], F32)

            lview = lrows.ap().rearrange("r (t p) d -> r p t d", p=128)

            for g in range(ROWS // GRP):
                s16 = s16pool.tile([GRP, 256], F32)
                xr_tiles = []
                for pp in range(GRP // 2):
                    ra = g * GRP + 2 * pp
                    j = 2 * pp
                    # ---- load + round both rows of the pair ----
                    xrs = []
                    for par in range(2):
                        x_f = xpool.tile([128, 2, D], F32)
                        nc.sync.dma_start(x_f[:], lview[ra + par])
                        xr = xrpool.tile([128, 2, D], F32R)
                        nc.vector.tensor_copy(xr[:], x_f[:])
                        xrs.append(xr)
                    xr_tiles.extend(xrs)

                    # ---- transpose X -> XT (pair layout [128,5,512]) ----
                    xt = xtpool.tile([128, 5, 512], F32R)
                    for (t0, t1) in ((0, 2), (2, 4), (4, 5)):
                        xtp = ps_xt.tile([128, 2, 512], F32)
                        for t in range(t0, t1):
                            for par in range(2):
                                for mt in range(2):
                                    nc.tensor.transpose(
                                        xtp[0:DSZ[t], t - t0,
                                            par * 256 + mt * 128:
                                            par * 256 + mt * 128 + 128
                                            ].bitcast(F32R),
                                        xrs[par][:, mt, DOF[t]:DOF[t] + DSZ[t]],
                                        idr[:],
                                    )
                        if t0 != 2:
                            nc.vector.tensor_copy(
                                xt[:, t0:t1, :], xtp[:, 0:t1 - t0, :])
                        else:
                            nc.scalar.copy(
                                xt[:, t0:t1, :], xtp[:, 0:t1 - t0, :])

                    if DEBUG_DUMPS and g == 0 and pp == 0:
                        nc.sync.dma_start(dbg_xt.ap(), xt[:].bitcast(F32))
                    # ---- L1: h1T[f, m-pair] ----
                    h1p = ps_h1.tile([128, 2, 512], F32)
                    for ft in range(2):
                        for t in range(5):
                            nc.tensor.matmul(
                                h1p[:, ft, :],
                                cw1r[0:DSZ[t], t, ft * 128:ft * 128 + 128],
                                xt[0:DSZ[t], t, :],
                                start=(t == 0), stop=(t == 4),
                            )
                    h1s = h1pool.tile([128, 2, 512], F32R)
                    for ft in range(2):
                        nc.scalar.activation(
                            h1s[:, ft, :], h1p[:, ft, :], AF.Relu,
                            bias=cb1_sb[:, ft:ft + 1], scale=1.0)

                    if DEBUG_DUMPS and g == 0 and pp == 0:
                        nc.sync.dma_start(dbg_h1.ap(), h1s[:].bitcast(F32))
                    # ---- L2: h2T [g, m-pair] ----
                    h2p = ps_h2.tile([128, 512], F32)
                    for ft in range(2):
                        nc.tensor.matmul(
                            h2p[:], cw2r[:, ft, :], h1s[:, ft, :],
                            start=(ft == 0), stop=(ft == 1))
                    h2s = h2pool.tile([128, 512], F32R)
                    nc.scalar.activation(h2s[:], h2p[:], AF.Relu,
                                         bias=cb2_sb[:, 0:1], scale=1.0)

                    if DEBUG_DUMPS and g == 0 and pp == 0:
                        nc.sync.dma_start(dbg_h2.ap(), h2s[:].bitcast(F32))
                    # ---- L3: scores for both rows in one N=512 matmul ----
                    sc = ps_rv.tile([1, 512], F32, tag="rv")
                    nc.tensor.matmul(sc[0:1, :], cw3r[:], h2s[:],
                                     start=True, stop=True)
                    sstage = smpool.tile([1, 2, 256], F32, tag="sstage")
                    nc.scalar.activation(
                        sstage[0:1, :, :], sc[0:1, :].rearrange("a (p f) -> a p f", p=2),
                        AF.Identity, bias=cb3_sb[0:1, 0:1], scale=1.0)
                    for par in range(2):
                        nc.sync.dma_start(s16[j + par:j + par + 1, :],
                                          sstage[0:1, par, :])

                if DEBUG_DUMPS and g == 0:
                    nc.sync.dma_start(dbg_s16.ap(), s16[:])
                # ---------------- group tail: softmax ----------------
                nmax = smpool.tile([GRP, 1], F32)
                nc.vector.tensor_reduce(nmax[:], s16[:], axis=AX.X,
                                        op=OP.max, negate=True)
                esum = smpool.tile([GRP, 1], F32)
                nc.scalar.activation(s16[:], s16[:], AF.Exp,
                                     bias=nmax[:], scale=1.0,
                                     accum_out=esum[:])
                rinv = smpool.tile([GRP, 1], F32)
                nc.vector.reciprocal(rinv[:], esum[:])
                nc.vector.tensor_scalar_mul(s16[:], in0=s16[:],
                                            scalar1=rinv[:])
                nc.sync.dma_start(w_out.ap()[g * GRP:(g + 1) * GRP, :],
                                  s16[:])

                # wT: [256, 16] -> gb[:, 0:32] (plain f32 transpose; the
                # psum->sbuf evac below is the f32r rounder)
                for mt in range(2):
                    nc.tensor.transpose(
                        gb[:, mt * 16:mt * 16 + 16],
                        s16[:, mt * 128:mt * 128 + 128],
                        id_f[0:GRP, 0:GRP])
                wt = smpool.tile([128, 32], F32R)
                nc.vector.tensor_copy(wt[:], gb[:, 0:32])

                # ---------------- matched / tgt_matched ----------------
                m16 = m16pool.tile([GRP, 528], F32)
                for j in range(GRP):
                    xr = xr_tiles[j]
                    if j % 2 == 0:
                        mstage = smpool.tile([1, 2, 528], F32, tag="mstage")
                    mv = ps_rv.tile([1, 512], F32, tag="rv")
                    for c in range(2):
                        for mt in range(2):
                            nc.tensor.matmul(
                                mv[0:1, c * 256:c * 256 + 256],
                                wt[:, mt * 16 + j:mt * 16 + j + 1],
                                xr[:, mt, c * 256:c * 256 + 256],
                                start=(mt == 0), stop=(mt == 1))
                    for mt in range(2):
                        nc.tensor.matmul(
                            gb[0:1, 144:156],
                            wt[:, mt * 16 + j:mt * 16 + j + 1],
                            xr[:, mt, 512:524],
                            start=(mt == 0), stop=(mt == 1))
                    for mt in range(2):
                        nc.tensor.matmul(
                            gb[0:1, 156:160],
                            wt[:, mt * 16 + j:mt * 16 + j + 1],
                            tgt_r[:, mt, :],
                            start=(mt == 0), stop=(mt == 1))
                    nc.scalar.activation(mstage[0:1, j % 2, 0:512],
                                         mv[0:1, :],
                                         AF.Identity, bias=0.0, scale=1.0)
                    nc.vector.tensor_copy(mstage[0:1, j % 2, 512:528],
                                          gb[0:1, 144:160])
                    if j % 2 == 1:
                        nc.sync.dma_start(m16[j - 1:j, :],
                                          mstage[0:1, 0, :])
                        nc.sync.dma_start(m16[j:j + 1, :],
                                          mstage[0:1, 1, :])

                # ---------------- overlap MLP (batched) ----------------
                for t in range(5):
                    nc.tensor.transpose(
                        gb[0:DSZ[t], 32 + t * 16:32 + t * 16 + 16],
                        m16[:, DOF[t]:DOF[t] + DSZ[t]],
                        id_f[0:GRP, 0:GRP])
                mtt = smpool.tile([128, 80], F32R)
                nc.vector.tensor_copy(mtt[:], gb[:, 32:112])
                for t in range(5):
                    nc.tensor.matmul(
                        gb[:, 112:128], ow1r[0:DSZ[t], t, :],
                        mtt[0:DSZ[t], t * 16:t * 16 + 16],
                        start=(t == 0), stop=(t == 4))
                h1o = smpool.tile([128, 16], F32R)
                nc.scalar.activation(h1o[:], gb[:, 112:128], AF.Relu,
                                     bias=ob1_sb[:, 0:1], scale=1.0)
                nc.tensor.matmul(gb[0:1, 128:144], ow2r[:], h1o[:],
                                 start=True, stop=True)
                nc.scalar.activation(
                    ov_sb[0:1, g * GRP:(g + 1) * GRP],
                    gb[0:1, 128:144], AF.Sigmoid,
                    bias=ob2_sb[0:1, 0:1], scale=1.0)

                nc.sync.dma_start(
                    tm_out.ap()[g * GRP:(g + 1) * GRP, :],
                    m16[:, 524:527])

            nc.sync.dma_start(ov_out.ap(), ov_sb[:])

    nc.compile()
    return nc


def _get_nc():
    global _CACHED_NC
    if _CACHED_NC is None:
        _CACHED_NC = build_nc()
    return _CACHED_NC


def make_in_maps(L_bat, tgt_centroids, cw1, cb1, cw2, cb2, cw3, cb3,
                 ow1, ob1, ow2, ob2):
    f = np.float32
    cw1p = np.zeros((5, 128, 256), f)
    for t in range(5):
        cw1p[t, 0:DSZ[t], :] = cw1[DOF[t]:DOF[t] + DSZ[t], :]
    ow1p = np.zeros((5, 128, 128), f)
    for t in range(5):
        ow1p[t, 0:DSZ[t], :] = ow1[DOF[t]:DOF[t] + DSZ[t], :]
    common = {
        "cw1p": cw1p,
        "cb1": np.ascontiguousarray(cb1.reshape(2, 128).T).astype(f),
        "cw2": np.ascontiguousarray(
            cw2.reshape(2, 128, 128).transpose(1, 0, 2)).astype(f),
        "cb2": cb2.reshape(128, 1).astype(f),
        "cw3": cw3.reshape(128, 1).astype(f),
        "cb3": cb3.reshape(1, 1).astype(f),
        "ow1p": ow1p,
        "ob1": ob1.reshape(128, 1).astype(f),
        "ow2": ow2.reshape(128, 1).astype(f),
        "ob2": ob2.reshape(1, 1).astype(f),
        "ident": np.eye(128, dtype=f),
    }
    lflat = L_bat.reshape(B * N, M, D)
    in_maps = []
    for k in range(NCORES):
        m = dict(common)
        m["lrows"] = np.ascontiguousarray(lflat[k * ROWS:(k + 1) * ROWS])
        tb = tgt_centroids[k // 2]  # [256, 3]
        tb4 = np.zeros((128, 2, 4), f)
        tb4[:, :, 0:3] = tb.reshape(2, 128, 3).transpose(1, 0, 2)
        m["tgtb"] = tb4
        in_maps.append(m)
    return in_maps


def run_device(in_maps, **kw):
    nc = _get_nc()
    res = run_bass_kernel_spmd(nc, in_maps, core_ids=list(range(NCORES)),
                               **kw)
    corr = np.concatenate(
        [res.results[k]["w_out"] for k in range(NCORES)], axis=0
    ).reshape(B, N, M)
    overlap = np.concatenate(
        [res.results[k]["ov_out"][0] for k in range(NCORES)], axis=0
    ).reshape(B, N, 1)
    tgt_matched = np.concatenate(
        [res.results[k]["tm_out"] for k in range(NCORES)], axis=0
    ).reshape(B, N, 3)
    return corr, overlap, tgt_matched, res


def _rot_mat_to_quat(Rm):
    tr = Rm[:, 0, 0] + Rm[:, 1, 1] + Rm[:, 2, 2]
    s0 = np.sqrt(np.maximum(tr + 1.0, 1e-10)) * 2
    q0 = np.stack([(Rm[:, 2, 1] - Rm[:, 1, 2]) / s0,
                   (Rm[:, 0, 2] - Rm[:, 2, 0]) / s0,
                   (Rm[:, 1, 0] - Rm[:, 0, 1]) / s0, 0.25 * s0], axis=-1)
    s1 = np.sqrt(np.maximum(1.0 + Rm[:, 0, 0] - Rm[:, 1, 1]
                            - Rm[:, 2, 2], 1e-10)) * 2
    q1 = np.stack([0.25 * s1, (Rm[:, 0, 1] + Rm[:, 1, 0]) / s1,
                   (Rm[:, 0, 2] + Rm[:, 2, 0]) / s1,
                   (Rm[:, 2, 1] - Rm[:, 1, 2]) / s1], axis=-1)
    s2 = np.sqrt(np.maximum(1.0 + Rm[:, 1, 1] - Rm[:, 0, 0]
                            - Rm[:, 2, 2], 1e-10)) * 2
    q2 = np.stack([(Rm[:, 0, 1] + Rm[:, 1, 0]) / s2, 0.25 * s2,
                   (Rm[:, 1, 2] + Rm[:, 2, 1]) / s2,
                   (Rm[:, 0, 2] - Rm[:, 2, 0]) / s2], axis=-1)
    s3 = np.sqrt(np.maximum(1.0 + Rm[:, 2, 2] - Rm[:, 0, 0]
                            - Rm[:, 1, 1], 1e-10)) * 2
    q3 = np.stack([(Rm[:, 0, 2] + Rm[:, 2, 0]) / s3,
                   (Rm[:, 1, 2] + Rm[:, 2, 1]) / s3, 0.25 * s3,
                   (Rm[:, 1, 0] - Rm[:, 0, 1]) / s3], axis=-1)
    c1 = (tr > 0)[:, None]
    c2 = ((Rm[:, 0, 0] > Rm[:, 1, 1])
          & (Rm[:, 0, 0] > Rm[:, 2, 2]))[:, None]
    c3 = (Rm[:, 1, 1] > Rm[:, 2, 2])[:, None]
    q = np.where(c1, q0, np.where(c2, q1, np.where(c3, q2, q3)))
    return q / np.maximum(np.linalg.norm(q, axis=-1, keepdims=True), 1e-12)


def host_tail(corr, overlap, tgt_matched, src_centroids):
    w = overlap[..., 0].astype(np.float64)
    src = src_centroids.astype(np.float64)
    tgtm = tgt_matched.astype(np.float64)
    w_norm = w / np.maximum(w.sum(axis=1, keepdims=True), EPS)
    src_bar = np.einsum('bn,bnc->bc', w_norm, src)
    tgt_bar = np.einsum('bn,bnc->bc', w_norm, tgtm)
    src_c = src - src_bar[:, None, :]
    tgt_c = tgtm - tgt_bar[:, None, :]
    H = np.einsum('bnc,bn,bnd->bcd', src_c, w_norm, tgt_c)
    U, S, Vh = np.linalg.svd(H)
    R_raw = np.einsum('bki,bjk->bij', Vh, U)
    det_sign = np.sign(np.linalg.det(R_raw))
    corr_fix = np.stack([np.ones_like(det_sign), np.ones_like(det_sign),
                         det_sign], axis=-1)
    Vh_fix = Vh * corr_fix[:, :, None]
    R = np.einsum('bki,bjk->bij', Vh_fix, U)
    translation = tgt_bar - np.einsum('bij,bj->bi', R, src_bar)
    quaternion = _rot_mat_to_quat(R)
    return (quaternion.astype(np.float32), translation.astype(np.float32),
            R.astype(np.float32))


def kernel(L_bat, src_centroids, tgt_centroids,
           cw1, cb1, cw2, cb2, cw3, cb3, ow1, ob1, ow2, ob2):
    args = [np.asarray(a, dtype=np.float32) for a in
            (L_bat, src_centroids, tgt_centroids, cw1, cb1, cw2, cb2,
             cw3, cb3, ow1, ob1, ow2, ob2)]
    (L_bat, src_centroids, tgt_centroids, cw1, cb1, cw2, cb2,
     cw3, cb3, ow1, ob1, ow2, ob2) = args
    in_maps = make_in_maps(L_bat, tgt_centroids, cw1, cb1, cw2, cb2,
                           cw3, cb3, ow1, ob1, ow2, ob2)
    corr, overlap, tgt_matched, _ = run_device(in_maps)
    quaternion, translation, R = host_tail(corr, overlap, tgt_matched,
                                           src_centroids)
    return (quaternion, translation, R,
            corr.astype(np.float32), overlap.astype(np.float32))
